# revision 54
# baseline (speedup 1.0000x reference)
"""NeuS renderer importance-sampling (up_sample step) on 8 Trainium2 cores.

Per ray (matching the jax reference): alpha/weights from SDF sigmoid CDF
differences + transmittance cumprod, then inverse-CDF sampling of 16
uniform mid-quantiles u_i=(i+0.5)/16 via the segment-sum identity
    result(b,i) = z[b,0] + sum_j dz_j * min(relu((u_i - cdf_{j+1})*r_j + 1), 1)
with r_j = 1/(cdf_{j+1}-cdf_j), clamped to BIG16 (step at cdf_{j+1}) when
the reference clips denom < 1e-5. Reproduces searchsorted+gather+lerp
without any gather.

Layout: 1024-ray tiles [128 partitions x 8 rays/row]. Work is spread
across DVE (fused custom ops), GPSIMD (tensor adds/subs), and ACT
(sigmoids/affines). Custom DVE ops carry a genuine 2X program in their
perf-mode table slot and declare perf_max=2 (2x_2p partition-pair mode;
fp32-safe). Empirically-derived engine constraints honored here: perf
slots execute correctly only for two-source scan-bearing programs (hence
the inert `+ scan(ADD, SrcN*0)` padding on elementwise ops), comparison-
free bodies for DENCS/RRFIX, no per-ray [P,R]->[P,R,S] broadcasts into
custom ops (those go to builtin/GPS ops), and no in-place custom ops.

When inv_s is a single constant (it is for this model's upsample step),
0.5*inv_s folds into the sigmoid activation scale and inv_s is neither
DMA'd nor touched; a general per-element path is kept as fallback.
"""

import numpy as np

B, S = 131072, 64
SM1 = S - 1
NCORES = 8
BC = B // NCORES
P = 128
R = 8
TILE_RAYS = P * R
NTILES = BC // TILE_RAYS
NI = 16
SIDE = 0.6
BIG16 = 60000.0  # rr cap; fits fp16, step width 1/BIG16 in u-space
SENT = -1.0      # clipped-segment sentinel in the rr chain

_CACHE = {}


def _register_ops():
    if "ops" in _CACHE:
        return _CACHE["ops"]
    from dataclasses import dataclass, field
    from concourse.dve_spec import (
        Spec, Src0, Src1, C0, C1, C2, One, Zero, relu, minn, maxx, eq, lower,
        AluOp, PageIdx, scan, select, Bin, _has_src1 as has_src1,
    )
    import concourse.dve_ops as dve_ops
    from concourse.dve_ops import DveOp, OPS
    from concourse.dve_uop import DveOpSpec, DveVer

    @dataclass(frozen=True)
    class DveOpPerf(DveOp):
        """Custom op whose table carries all four perf-mode programs."""

        perf_max_decl: int = 0

        def compile(self, ver: DveVer) -> DveOpSpec:
            key = ("__perf__" + self.name, ver)
            cache = dve_ops._COMPILE_CACHE
            if (r := cache.get(key)) is not None:
                return r
            uops = lower(self.spec, ver=ver)
            # 2x slot holds a real program; 2x_2p/4x legally fall back to it
            # (table-gen writes reuse entries), keeping the control table small.
            result = DveOpSpec(
                name=self.name,
                opcode=dve_ops.get_dve_sub_opcode(self.name),
                uops=uops,
                rd1_en=has_src1(self.spec),
                uops_2x=list(uops) if self.perf_max_decl >= 1 else None,
                perf_max=self.perf_max_decl,
            )
            cache[key] = result
            return result

    def mk(name, spec, subdim, perf):
        for op in OPS:
            if op.name == name:
                return op
        op = DveOpPerf(name, spec, subdim=subdim, uops_sha={},
                       perf_max_decl=perf)
        OPS.append(op)
        dve_ops.CUSTOM_DVE_SPECS[name] = spec
        dve_ops._SUB_OPCODE_FOR_NAME[name] = dve_ops._CUSTOM_DVE_ROW_BASE + len(OPS) - 1
        assert dve_ops._SUB_OPCODE_FOR_NAME[name] < 0x20
        return op

    def pg_arr(in0, c0, c1):
        sd = int(np.prod(in0.shape[1:-1]))
        base = np.asarray(c0, np.float32).reshape(-1, 1) if isinstance(c0, np.ndarray) else c0
        st = float(c1.flat[0]) if isinstance(c1, np.ndarray) else c1
        idx = base + st * np.arange(sd, dtype=np.float32)[None, :]
        return idx[..., None]  # [1|P, sd, 1]

    def f32(a):
        return np.asarray(a, np.float32) if a is not None else None

    def bc0(c0, nd):
        return (np.asarray(c0, np.float32).reshape(-1, *([1] * (nd - 1)))
                if isinstance(c0, np.ndarray) else c0)

    def n2(in0, in1):
        """Flatten both operands to a common [P, F] shape (the interp may
        coalesce one AP's free dims but not a broadcast view's)."""
        a = f32(in0).reshape(in0.shape[0], -1)
        b = f32(in1).reshape(in1.shape[0], -1)
        return a, b

    # --- references (these define interp semantics; fp32 internally) ---
    def ref_tsel(in0, in1, c0, c1, c2):
        i0 = f32(in0)
        pg = pg_arr(in0, c0, c1)
        return np.minimum(np.maximum((pg - f32(in1)) * i0 + 1.0, 0.0), 1.0).reshape(in0.shape)

    def ref_mulscan(in0, in1, c0, c1, c2):
        a = f32(in0).reshape(in0.shape[0], -1)
        b = f32(in1).reshape(in1.shape[0], -1)
        return np.cumsum(a * b, -1, dtype=np.float32).reshape(in0.shape)

    def ref_max2(in0, in1, c0, c1, c2):
        a, b = n2(in0, in1)
        return np.maximum(a, b).reshape(in0.shape)

    def ref_mul2(in0, in1, c0, c1, c2):
        a, b = n2(in0, in1)
        return (a * b).reshape(in0.shape)

    def ref_minclip(in0, in1, c0, c1, c2):
        a, b = n2(in0, in1)
        return np.minimum(np.maximum(np.minimum(a, b), bc0(c0, 2)), 0.0).reshape(in0.shape)

    def ref_muladd(in0, in1, c0, c1, c2):
        a, b = n2(in0, in1)
        return (a * b + bc0(c0, 2)).reshape(in0.shape)

    def ref_negmuladd(in0, in1, c0, c1, c2):
        # (c1 - in0) * in1 + c0
        a, b = n2(in0, in1)
        return ((bc0(c1, 2) - a) * b + bc0(c0, 2)).reshape(in0.shape)

    def ref_cumprod2(in0, in1, c0, c1, c2):
        a, b = n2(in0, in1)
        return np.cumprod(a * b, -1, dtype=np.float32).reshape(in0.shape)

    def ref_cumprod3(in0, in1, c0, c1, c2):
        a, b = n2(in0, in1)
        return np.cumprod(a * b + np.float32(c2), -1, dtype=np.float32).reshape(in0.shape)

    def ref_scalemulsub(in0, in1, c0, c1, c2):
        # a*c1 - b + c0
        a, b = n2(in0, in1)
        return (a * bc0(c1, 2) - b + bc0(c0, 2)).reshape(in0.shape)

    def ref_dencs(in0, in1, c0, c1, c2):
        # a + (min(a,c0)-c0)*c1: identity for a >= c0, large-negative below
        a = f32(in0)
        return a + (np.minimum(a, bc0(c0, a.ndim)) - bc0(c0, a.ndim)) * bc0(c1, a.ndim)

    def ref_rrfix(in0, in1, c0, c1, c2):
        # min(max(a, -a*c0), c1): passes a>0 through, maps a<0 to the c1 cap
        a = f32(in0)
        return np.minimum(np.maximum(a, -a * bc0(c0, a.ndim)), bc0(c1, a.ndim))

    def ref_recipf(in0, in1, c0, c1, c2):
        not_x = (~f32(in0).view(np.int32)).view(np.float32)
        y0 = not_x * np.float32(-0.23549792)
        y1 = y0 * (np.float32(2.0017324) - f32(in0) * y0)
        return y1 * (np.float32(2.0) - f32(in0) * y1)

    def ref_recipf2(in0, in1, c0, c1, c2):
        # one-NR variant (~0.4% rel err): for tolerance-insensitive uses
        not_x = (~f32(in0).view(np.int32)).view(np.float32)
        y0 = not_x * np.float32(-0.23549792)
        return y0 * (np.float32(2.0017324) - f32(in0) * y0)


    _rf_not = Bin(AluOp.BITWISE_NOT, Src0, Src0)
    _rf_y0 = _rf_not * C0
    _rf_y1 = _rf_y0 * (C1 - Src0 * _rf_y0)


    pg = PageIdx(C0, C1)

    def spad(body):
        """Append a zero-valued scan term: keeps semantics, makes the uop
        program scan-bearing/multi-state (required for perf-mode slots to
        execute correctly on the engine)."""
        return body + scan(AluOp.ADD, Src0 * Zero)

    def spad2(body):
        """spad variant that also reads Src1 (perf-mode slots additionally
        require two-source programs); call with in1= any partition-matched
        tensor (in0 itself works)."""
        return body + scan(AluOp.ADD, Src1 * Zero)

    ops = {
        "T_SEL": mk("T_SEL_ANT", Spec(
            body=minn(relu((pg - Src1) * Src0 + One), One), reference=ref_tsel),
            True, 2),
        "MULSCAN": mk("MULSCAN_ANT", Spec(
            body=scan(AluOp.ADD, Src0 * Src1), reference=ref_mulscan), False, 2),
        "MAX2": mk("MAX2_ANT", Spec(
            body=spad(maxx(Src0, Src1)), reference=ref_max2), False, 2),
        "MUL2": mk("MUL2_ANT", Spec(
            body=spad(Src0 * Src1), reference=ref_mul2), False, 2),
        "MINCLIP": mk("MINCLIP_ANT", Spec(
            body=spad(minn(maxx(minn(Src0, Src1), C0), Zero)), reference=ref_minclip),
            False, 2),
        "MULADD": mk("MULADD_ANT", Spec(
            body=spad(Src0 * Src1 + C0), reference=ref_muladd), False, 2),
        "NEGMULADD": mk("NEGMULADD_ANT", Spec(
            body=spad((C1 - Src0) * Src1 + C0), reference=ref_negmuladd), False, 2),
        "CUMPROD2": mk("CUMPROD2_ANT", Spec(
            body=scan(AluOp.MULTIPLY, Src0 * Src1), reference=ref_cumprod2),
            False, 2),
        "CUMPROD3": mk("CUMPROD3_ANT", Spec(
            body=scan(AluOp.MULTIPLY, Src0 * Src1 + C2), reference=ref_cumprod3),
            False, 2),
        "SCALEMULSUB": mk("SCALEMULSUB_ANT", Spec(
            body=spad(Src0 * C1 - Src1 + C0), reference=ref_scalemulsub),
            False, 2),
        "DENCS": mk("DENCS_ANT", Spec(
            body=spad2(Src0 + (minn(Src0, C0) - C0) * C1),
            reference=ref_dencs), False, 2),
        "RRFIX": mk("RRFIX_ANT", Spec(
            body=spad2(minn(maxx(Src0, (Zero - Src0) * C0), C1)),
            reference=ref_rrfix), False, 2),
        "RECIPF": mk("RECIPF_ANT", Spec(
            body=_rf_y1 * (C2 - Src0 * _rf_y1), reference=ref_recipf), False, 0),
        "RECIPF2": mk("RECIPF2_ANT", Spec(
            body=spad2(_rf_y1), reference=ref_recipf2), False, 2),
    }
    _CACHE["ops"] = ops
    return ops


def _build_program(scale_const):
    """scale_const: float (0.5*inv_s folded into sigmoid scale) or None for
    the general per-element inv_s path."""
    import os
    BISECT = int(os.environ.get("KBISECT", "99"))
    NOCUST = bool(int(os.environ.get("KNOCUST", "0")))
    PERF0 = bool(int(os.environ.get("KPERF0", "0")))
    PERF_ONLY = os.environ.get("KPERFONLY")  # comma-sep op keys, or None=all
    import concourse.bass as bass
    import concourse.mybir as mybir
    from concourse import bacc
    from concourse.tile import TileContext

    OPS = _register_ops()
    f32 = mybir.dt.float32
    f16 = mybir.dt.float16
    Alu = mybir.AluOpType
    Act = mybir.ActivationFunctionType
    Ax = mybir.AxisListType

    RF = {"s0": -0.23549792, "s1": 2.0017324, "imm2": 2.0}

    nc = bacc.Bacc()
    z_d = nc.declare_dram_parameter("z_vals", [BC, S], f32, isOutput=False)
    s_d = nc.declare_dram_parameter("sdf", [BC, S], f32, isOutput=False)
    if scale_const is None:
        i_d = nc.declare_dram_parameter("inv_s", [BC, SM1], f32, isOutput=False)
    o_d = nc.declare_dram_parameter("rays_o", [BC, 3], f32, isOutput=False)
    d_d = nc.declare_dram_parameter("rays_d", [BC, 3], f32, isOutput=False)
    out_d = nc.declare_dram_parameter("out", [BC, NI], f32, isOutput=True)

    V = nc.vector
    A = nc.scalar
    G = nc.gpsimd

    def cop(op, out, in0, in1=None, s0=0.0, s1=0.0, imm2=0.0, perf=None):
        bi = V._custom_dve(OPS[op], out=out, in0=in0, in1=in1, s0=s0, s1=s1,
                           imm2=imm2)
        pm = OPS[op].perf_max_decl if perf is None else perf
        if PERF0 or (PERF_ONLY is not None and op not in PERF_ONLY.split(",")):
            pm = 0
        bi.ins.perf_max = pm
        return bi

    with TileContext(nc) as tc, \
         tc.tile_pool(name="const", bufs=1) as cp, \
         tc.tile_pool(name="io", bufs=3) as io, \
         tc.tile_pool(name="wk", bufs=3) as wk, \
         tc.tile_pool(name="w16", bufs=2) as w16p, \
         tc.tile_pool(name="ybuf", bufs=2) as yp:

        outb = cp.tile([P, NTILES * R * NI], f32)
        outv = outb[:].rearrange("p (t r n) -> p t r n", t=NTILES, r=R)
        ones = cp.tile([P, 1], f32)
        V.memset(ones[:], 1.0)
        eps7 = cp.tile([P, 1], f32)
        V.memset(eps7[:], 1e-7)

        def emit_phase2(pt, prrv, pcdfv, pdz, pV_z):
            # selection: X = clamp((u - cdf_{j+1})*rr + 1, 0, 1); emitted one
            # tile late so DVE has data-ready work while GPS/ACT run the next
            # tile's phase 1 (software pipelining).
            bnd = wk.tile([P, R * (NI + 1)], f32, tag="bnd")
            bnd3 = bnd[:].rearrange("p (r n) -> p r n", r=R)
            V.memset(bnd3[:, :, 0:1], 0.0)
            xa = w16p.tile([P, NI * SM1], f32, tag="Xa")
            xb = w16p.tile([P, NI * SM1], f32, tag="Xb")
            ya = yp.tile([P, NI * SM1], f32, tag="Ya")
            yb = yp.tile([P, NI * SM1], f32, tag="Yb")
            Xpair = [xa, xb]
            Ypair = [ya, yb]
            for r in range(R):
                X3 = Xpair[r % 2][:].rearrange("p (n s) -> p n s", n=NI)
                rr_b = prrv[:, r, :].unsqueeze(1).broadcast_to((P, NI, SM1))
                cdf_b = pcdfv[:, r, :].unsqueeze(1).broadcast_to((P, NI, SM1))
                cop("T_SEL", out=X3, in0=rr_b, in1=cdf_b,
                    s0=0.5 / NI, s1=1.0 / NI)
                Y3 = Ypair[r % 2][:].rearrange("p (n s) -> p n s", n=NI)
                dzr_b = pdz[:, r, :].unsqueeze(1).broadcast_to((P, NI, SM1))
                cop("MULSCAN", out=Y3, in0=X3, in1=dzr_b)
                A.activation(bnd3[:, r, 1:NI + 1], Y3[:, :, SM1 - 1], Act.Copy)
            res3 = outv[:, pt, :, :]
            G.tensor_sub(res3, bnd3[:, :, 1:NI + 1], bnd3[:, :, 0:NI])
            z0_b = pV_z[:, :, 0:1].broadcast_to((P, R, NI))
            G.tensor_add(res3, res3, z0_b)

        pend = []
        for t in range(NTILES):
            rows = slice(t * TILE_RAYS, (t + 1) * TILE_RAYS)

            zt = io.tile([P, R * S], f32, tag="zt")
            V_z = zt[:].rearrange("p (r s) -> p r s", r=R)
            nc.sync.dma_start(out=V_z, in_=z_d.ap()[rows, :].rearrange("(p r) s -> p r s", p=P))
            st = io.tile([P, R * S], f32, tag="st")
            V_s = st[:].rearrange("p (r s) -> p r s", r=R)
            nc.sync.dma_start(out=V_s, in_=s_d.ap()[rows, :].rearrange("(p r) s -> p r s", p=P))
            if scale_const is None:
                it = io.tile([P, R * SM1], f32, tag="it")
                V_i = it[:].rearrange("p (r s) -> p r s", r=R)
                nc.sync.dma_start(out=V_i,
                                  in_=i_d.ap()[rows, :].rearrange("(p r) s -> p r s", p=P))
            ot = io.tile([P, R * 3], f32, tag="ot")
            nc.sync.dma_start(out=ot[:].rearrange("p (r c) -> p r c", r=R),
                              in_=o_d.ap()[rows, :].rearrange("(p r) c -> p r c", p=P))
            dt_ = io.tile([P, R * 3], f32, tag="dt")
            nc.sync.dma_start(out=dt_[:].rearrange("p (r c) -> p r c", r=R),
                              in_=d_d.ap()[rows, :].rearrange("(p r) c -> p r c", p=P))

            if BISECT == 10:
                V.memset(outv[:, t, :, :], 0.0)
                continue

            if len(pend) > 1:
                emit_phase2(*pend.pop(0))

            def w64(tag):
                tl = wk.tile([P, R * S], f32, tag=tag)
                return tl[:].rearrange("p (r s) -> p r s", r=R)

            def w63(tag):
                return w64(tag)[:, :, 0:SM1]

            # ---- per-ray slab interval [lo, hi] (tiny 24/8-free ops, DVE) --
            rdt = wk.tile([P, R * 3], f32, tag="rd")
            V.reciprocal(rdt[:], dt_[:])
            V_rd = rdt[:].rearrange("p (r c) -> p r c", r=R)
            t1 = wk.tile([P, R * 3], f32, tag="t1")
            V.tensor_scalar(t1[:], ot[:], SIDE, -1.0, Alu.subtract, Alu.mult)
            t1m = wk.tile([P, R * 3], f32, tag="t1m")
            if NOCUST:
                V.tensor_tensor(t1m[:], t1[:], rdt[:], Alu.mult)
            else:
                cop("MUL2", out=t1m[:], in0=t1[:], in1=rdt[:])
            t2 = wk.tile([P, R * 3], f32, tag="t2")
            V.tensor_scalar(t2[:], ot[:], -SIDE, -1.0, Alu.subtract, Alu.mult)
            t2m = wk.tile([P, R * 3], f32, tag="t2m")
            if NOCUST:
                V.tensor_tensor(t2m[:], t2[:], rdt[:], Alu.mult)
            else:
                cop("MUL2", out=t2m[:], in0=t2[:], in1=rdt[:])
            loc = wk.tile([P, R * 3], f32, tag="loc")
            hic = wk.tile([P, R * 3], f32, tag="hic")
            V.tensor_tensor(loc[:], t1m[:], t2m[:], Alu.min)
            V.tensor_tensor(hic[:], t1m[:], t2m[:], Alu.max)
            lo = wk.tile([P, R], f32, tag="lo")
            hi = wk.tile([P, R], f32, tag="hi")
            V.tensor_reduce(lo[:], loc[:].rearrange("p (r c) -> p r c", r=R), axis=Ax.X, op=Alu.max)
            V.tensor_reduce(hi[:], hic[:].rearrange("p (r c) -> p r c", r=R), axis=Ax.X, op=Alu.min)
            lo_b = lo[:].unsqueeze(2).broadcast_to((P, R, S))
            hi_b = hi[:].unsqueeze(2).broadcast_to((P, R, S))
            if BISECT == 11:
                V.memset(outv[:, t, :, :], 0.0)
                continue

            # ---- inside mask ----
            m1 = w64("m1")
            V.tensor_tensor(m1, V_z, lo_b, Alu.is_ge)
            m2 = w64("m2")
            V.tensor_tensor(m2, V_z, hi_b, Alu.is_le)
            if BISECT == 12:
                V.memset(outv[:, t, :, :], 0.0)
                continue
            inner = w64("inner")
            G.tensor_mul(inner, m1, m2)
            inside = w63("inside")
            cop("MAX2", out=inside, in0=inner[:, :, 0:SM1], in1=inner[:, :, 1:S])

            if BISECT == 13:
                V.memset(outv[:, t, :, :], 0.0)
                continue
            # ---- GPS lane: diffs/sums ----
            dz = w63("dz")
            G.tensor_sub(dz, V_z[:, :, 1:S], V_z[:, :, 0:SM1])
            sdiff = w63("sdiff")
            G.tensor_sub(sdiff, V_s[:, :, 1:S], V_s[:, :, 0:SM1])
            ssum = w63("ssum")
            G.tensor_add(ssum, V_s[:, :, 0:SM1], V_s[:, :, 1:S])

            # ---- cos chain ----
            dzeps = w63("dzeps")
            A.activation(dzeps, dz, Act.Copy, bias=1e-5)
            rdze = w63("rdze")
            cop("RECIPF2", out=rdze, in0=dzeps, in1=dzeps, s0=RF["s0"], s1=RF["s1"])
            if BISECT == 14:
                V.memset(outv[:, t, :, :], 0.0)
                continue
            idz = w63("idz")
            G.tensor_mul(idz, inside, dz)
            cosb = w64("cosb")
            V.memset(cosb[:, :, 0:1], 0.0)
            G.tensor_mul(cosb[:, :, 1:S], sdiff, rdze)
            cosm = w63("cosm")
            cop("MINCLIP", out=cosm, in0=cosb[:, :, 0:SM1], in1=cosb[:, :, 1:S],
                s0=-1000.0)
            cd = w63("cd")
            G.tensor_mul(cd, cosm, idz)
            if BISECT <= 2:
                V.memset(outv[:, t, :, :], 0.0)
                continue

            # ---- sigmoid args ----
            parg = w63("parg")
            G.tensor_sub(parg, ssum, cd)
            narg = w63("narg")
            G.tensor_add(narg, ssum, cd)
            if scale_const is None:
                halfs = w63("halfs")
                A.activation(halfs, V_i, Act.Copy, scale=0.5)
                G.tensor_mul(parg, parg, halfs)
                G.tensor_mul(narg, narg, halfs)
                sig_scale = 1.0
            else:
                sig_scale = float(scale_const)
            pcdf = w63("pcdf")
            A.activation(pcdf, parg, Act.Sigmoid, scale=sig_scale)
            ncdf = w63("ncdf")
            A.activation(ncdf, narg, Act.Sigmoid, scale=sig_scale)
            if BISECT <= 3:
                V.memset(outv[:, t, :, :], 0.0)
                continue

            # ---- alpha/trans/weights ----
            pde = w63("pde")
            A.activation(pde, pcdf, Act.Copy, bias=1e-5)
            rpde = w63("rpde")
            cop("RECIPF", out=rpde, in0=pde, s0=RF["s0"], s1=RF["s1"], imm2=RF["imm2"])
            trb = w64("trb")
            V.memset(trb[:, :, 0:1], 1.0)
            for r in range(R):
                cop("CUMPROD3", out=trb[:, r, 1:S], in0=ncdf[:, r, :],
                    in1=rpde[:, r, :], imm2=1e-7)
            wp = w63("wp")
            cop("SCALEMULSUB", out=wp, in0=trb[:, :, 0:SM1], in1=trb[:, :, 1:S],
                s0=1e-5, s1=1.0 + 1e-7)
            if BISECT <= 4:
                V.memset(outv[:, t, :, :], 0.0)
                continue

            # ---- cdf (normalized, fp16) + rr (fp16) ----
            cwp = w63("cwp")
            ones_s1 = ones[:].broadcast_to((P, SM1))
            for r in range(R):
                cop("MULSCAN", out=cwp[:, r, :], in0=wp[:, r, :], in1=ones_s1)
            tott = wk.tile([P, R], f32, tag="tot")
            V.tensor_copy(tott[:], cwp[:, :, SM1 - 1])
            rtot = wk.tile([P, R], f32, tag="rtot")
            cop("RECIPF", out=rtot[:], in0=tott[:], s0=RF["s0"], s1=RF["s1"], imm2=RF["imm2"])
            rtot_b = rtot[:].unsqueeze(2).broadcast_to((P, R, SM1))
            pdf = w63("pdf")
            G.tensor_mul(pdf, wp, rtot_b)
            cdft = wk.tile([P, R * SM1], f32, tag="cdf")
            cdfv = cdft[:].rearrange("p (r s) -> p r s", r=R)
            G.tensor_mul(cdfv, cwp, rtot_b)

            denc = w63("dzeps")  # reuse
            cop("DENCS", out=denc, in0=pdf, in1=pdf, s0=1e-5, s1=1e9)
            rrf = w63("rdze")  # reuse
            cop("RECIPF2", out=rrf, in0=denc, in1=denc, s0=RF["s0"], s1=RF["s1"])
            rrt = wk.tile([P, R * SM1], f32, tag="rr")
            rrv = rrt[:].rearrange("p (r s) -> p r s", r=R)
            cop("RRFIX", out=rrv, in0=rrf, in1=rrf, s0=1e9, s1=BIG16)
            if BISECT <= 5:
                V.memset(outv[:, t, :, :], 0.0)
                continue

            pend.append((t, rrv, cdfv, dz, V_z))

        while pend:
            emit_phase2(*pend.pop(0))

        nc.sync.dma_start(
            out=out_d.ap().rearrange("(t p r) n -> p t r n", t=NTILES, p=P),
            in_=outv,
        )

    nc.compile()
    return nc


def _get_nc(scale_const=32.0):
    key = ("nc", scale_const)
    if key not in _CACHE:
        _CACHE[key] = _build_program(scale_const)
    return _CACHE[key]


def kernel(rays_o, rays_d, z_vals, sdf, inv_s, n_importance):
    from concourse.bass_utils import run_bass_kernel_spmd

    assert int(n_importance) == NI
    inv_s = np.asarray(inv_s, np.float32)
    const_val = float(inv_s.flat[0])
    is_const = bool(np.all(inv_s == const_val))
    scale_const = 0.5 * const_val if is_const else None
    nc = _get_nc(scale_const)
    in_maps = []
    for c in range(NCORES):
        rows = slice(c * BC, (c + 1) * BC)
        m = {
            "z_vals": np.ascontiguousarray(z_vals[rows]),
            "sdf": np.ascontiguousarray(sdf[rows]),
            "rays_o": np.ascontiguousarray(rays_o[rows]),
            "rays_d": np.ascontiguousarray(rays_d[rows]),
        }
        if scale_const is None:
            m["inv_s"] = np.ascontiguousarray(inv_s[rows])
        in_maps.append(m)
    res = run_bass_kernel_spmd(nc, in_maps, list(range(NCORES)))
    return np.concatenate([res.results[c]["out"] for c in range(NCORES)], axis=0)
